# revision 26
# baseline (speedup 1.0000x reference)
"""Grouped-Query Attention kernel for Trainium2 (8 NeuronCores, SPMD).

Problem: x [4, 4096, 512] fp32, per-group Dense Q/K/V (G=4 groups of 128
features), full softmax attention within each (batch, group) pair, output
re-concatenated to [4, 4096, 512].

Sharding: B*G = 16 fully independent attention problems -> 2 per core.

v2 design (software-pipelined, fp16 compute):
  - prologue: load xg, cast fp16, PE-transpose to xgT [d, t];
    Q^T/K^T = W^T xgT (bias folded); V computed NATURAL directly via
    matmul(lhsT=xgT_chunk, rhs=Wv) with bv folded in (exact: softmax
    weights sum to 1, so out = P(V+bv)/den == PV/den + bv).
  - scores transposed: S^T[ts, tq] = K^T(lhsT) @ Q^T chunks; exp via ACT
    with 1/sqrt(gs) folded into the scale; probabilities pt land in fp16.
  - denominator: pt chunk pairs summed on DVE (fp16 2x mode), then 8
    one-column matmuls (lhsT=pair slice, rhs=ones) accumulate den^T[tq,1]
    directly in natural orientation -> tiny [128,8] PSUM tile per macro.
  - out^T accumulated over ts chunks in PSUM; epilogue per macro: DVE
    drains ps_out -> SBUF, approx-fast reciprocal of [128,8] den,
    PE-transpose 128-blocks to natural, per-partition tensor_scalar_mul
    by recip, DMA out.
  - emission is software-pipelined: out-matmuls lag 2 chunks behind
    scores/exp; the previous macro's epilogue is spread over slots in the
    first chunks of the next macro. PE never han an intentional idle slot
    (TRN2 PE drops to half clock after any idle gap).
"""

import os
import sys

sys.path.insert(0, "/opt/trn_rl_repo")

import numpy as np

import concourse.bass as bass
import concourse.mybir as mybir
import concourse.tile as tile
from concourse.masks import make_identity

B, T, F, G = 4, 4096, 512, 4
GS = F // G  # 128
N_CORES = 8
PAIRS_PER_CORE = (B * G) // N_CORES  # 2
TQ = 1024  # query tile width per macro
NM = T // TQ  # 4 macros
NCH = T // 128  # 32 key/time chunks
INV_SCALE = float(1.0 / (np.sqrt(np.float32(GS)) + 1e-9))

FP32 = mybir.dt.float32
FP16 = mybir.dt.float16

_NC_CACHE = None
_LAST_IN_MAPS = None


def _split_multi_waits(nc):
    """Walrus codegen rejects instructions carrying more than one semaphore
    wait on several instruction structs (DMA DIRECT2D, tensor_scalar, LDW).
    Hoist all-but-the-last wait of any multi-wait instruction onto same-engine
    NoOps inserted immediately before it: the sequencer executes them in
    order, so the gating semantics are identical."""
    n_split = 0
    for func in nc.m.functions:
        for block in func.blocks:
            new = []
            for inst in block.instructions:
                si = inst.sync_info
                waits = list(si.on_wait) if (si is not None and si.on_wait) else []
                if len(waits) > 1:
                    for w in waits[:-1]:
                        nop = mybir.InstNoOp(
                            name=nc.get_next_instruction_name(), ins=[], outs=[]
                        )
                        nop.engine = inst.engine
                        nop.sync_info = mybir.SyncInfo(on_wait=[w], on_update=[])
                        new.append(nop)
                        n_split += 1
                    inst.sync_info = mybir.SyncInfo(
                        on_wait=[waits[-1]],
                        on_update=list(si.on_update) if si.on_update else [],
                    )
                new.append(inst)
            block.instructions = new
    return n_split


def build_nc():
    nc = bass.Bass()

    ins = []
    outs = []
    outs_dbg = None
    if os.environ.get("DBG_DEN") == "1":
        outs_dbg = nc.declare_dram_parameter("dbg0", [128, 16], FP32, isOutput=True)
    for i in range(PAIRS_PER_CORE):
        ins.append(
            dict(
                x=nc.declare_dram_parameter(f"x{i}", [T, GS], FP32, isOutput=False),
                wq=nc.declare_dram_parameter(f"wq{i}", [GS, GS], FP32, isOutput=False),
                wk=nc.declare_dram_parameter(f"wk{i}", [GS, GS], FP32, isOutput=False),
                wv=nc.declare_dram_parameter(f"wv{i}", [GS, GS], FP32, isOutput=False),
                bq=nc.declare_dram_parameter(f"bq{i}", [1, GS], FP32, isOutput=False),
                bk=nc.declare_dram_parameter(f"bk{i}", [1, GS], FP32, isOutput=False),
                bv=nc.declare_dram_parameter(f"bv{i}", [1, GS], FP32, isOutput=False),
            )
        )
        outs.append(nc.declare_dram_parameter(f"y{i}", [T, GS], FP32, isOutput=True))

    with tile.TileContext(nc) as tc:
        with (
            tc.tile_pool(name="consts", bufs=1) as consts,
            tc.tile_pool(name="bigsb", bufs=2) as bigsb,  # per-pair persistent
            tc.tile_pool(name="ptp", bufs=4) as ptp,  # exp'd prob chunks
            tc.tile_pool(name="ppair", bufs=2) as ppair,  # chunk-pair sums
            tc.tile_pool(name="epi", bufs=2) as epi,  # epilogue sbuf tiles
            tc.tile_pool(name="ps", bufs=2, space="PSUM") as ps,  # scores + misc
            tc.tile_pool(name="ps_o", bufs=1, space="PSUM") as ps_o,  # out^T
            tc.tile_pool(name="ps_dn", bufs=1, space="PSUM") as ps_dn,  # den^T
            tc.tile_pool(name="ps_ep", bufs=1, space="PSUM") as ps_ep,  # nat out
        ):
            ident_h = consts.tile([128, 128], FP16)
            make_identity(nc, ident_h)
            ident_f = consts.tile([128, 128], FP32)
            make_identity(nc, ident_f)
            ones_col = consts.tile([128, 1], FP16)
            nc.vector.memset(ones_col, 1.0)

            # ------------- input DMAs for both pairs up front -------------
            # x arrives in 4 slab DMAs so casting/transposing can start as
            # soon as the first slab lands; weight DMAs precede bias DMAs so
            # the weight casts aren't gated behind the whole DMA queue.
            pair_in = []
            for i in range(PAIRS_PER_CORE):
                p = ins[i]
                xg_f = bigsb.tile([128, NCH, 128], FP32, tag="xg_f")
                for k in range(4):
                    nc.sync.dma_start(
                        out=xg_f[:, k * 8 : (k + 1) * 8, :],
                        in_=p["x"][k * 1024 : (k + 1) * 1024, :].rearrange(
                            "(c p) d -> p c d", p=128
                        ),
                    )
                pair_in.append(dict(xg_f=xg_f))
            for i in range(PAIRS_PER_CORE):
                p = ins[i]
                w_f = {}
                for nm in ("wq", "wk", "wv"):
                    wf = consts.tile([128, 128], FP32, tag=f"wf_{nm}{i}")
                    nc.gpsimd.dma_start(out=wf, in_=p[nm][:, :])
                    w_f[nm] = wf
                pair_in[i]["w_f"] = w_f
            for i in range(PAIRS_PER_CORE):
                p = ins[i]
                b_col = {}
                for nm in ("bq", "bk"):
                    bc = consts.tile([128, 1], FP32, tag=f"{nm}{i}")
                    nc.gpsimd.dma_start(
                        out=bc, in_=p[nm][:, :].rearrange("o d -> d o")
                    )
                    b_col[nm] = bc
                bvb = consts.tile([128, 128], FP32, tag=f"bvb{i}")
                _bv = p["bv"][:, :]
                nc.gpsimd.dma_start(
                    out=bvb,
                    in_=bass.AP(tensor=_bv.tensor, offset=_bv.offset,
                                ap=[[0, 128]] + list(_bv.ap[1:])),
                )
                pair_in[i]["b_col"] = b_col
                pair_in[i]["bvb"] = bvb

            # ------------- prologues -------------
            # Only pair 0's xgT section plus the first QK/V tiles run as a
            # standalone prologue.  Everything else (pair0's remaining QK/V
            # tiles and pair1's whole prologue) is emitted as "fillers", one
            # per attention chunk slot, so that work runs inside the hot PE
            # stream (full clock, no drain-latency stalls).
            pair_data = []
            for i in range(PAIRS_PER_CORE):
                pd = dict(
                    xg_h=None, xgT=None,
                    qt=None, kt=None, v_nat=None,
                    w_h={}, bvb=pair_in[i]["bvb"],
                )
                pair_data.append(pd)

            def emit_w_casts(i):
                pd, pi = pair_data[i], pair_in[i]
                for nm in ("wq", "wk", "wv"):
                    wh = consts.tile([128, 128], FP16, tag=f"wh_{nm}{i}", name="wh")
                    nc.vector.tensor_copy(wh, pi["w_f"][nm])
                    pd["w_h"][nm] = wh

            def emit_slab_cast(i, k):
                pd, pi = pair_data[i], pair_in[i]
                if pd["xg_h"] is None:
                    pd["xg_h"] = bigsb.tile([128, NCH, 128], FP16, tag="xg_h", name="xg_h")
                    pd["xgT"] = bigsb.tile([128, T], FP16, tag="xgT", name="xgT")
                nc.vector.tensor_copy(
                    pd["xg_h"][:, k * 8 : (k + 1) * 8, :],
                    pi["xg_f"][:, k * 8 : (k + 1) * 8, :],
                )

            def emit_T_group(i, g, dve_only):
                pd = pair_data[i]
                c0 = g * 4
                pst = ps.tile([128, 512], FP16, tag="sc", name="pst")
                for q in range(4):
                    nc.tensor.transpose(
                        pst[:, q * 128 : (q + 1) * 128],
                        pd["xg_h"][:, c0 + q, :], ident_h,
                    )
                dsl = slice(c0 * 128, (c0 + 4) * 128)
                if dve_only or g % 2 == 0:
                    nc.vector.tensor_copy(pd["xgT"][:, dsl], pst)
                else:
                    nc.scalar.copy(pd["xgT"][:, dsl], pst)

            def emit_qk_tile(i, which, jj, dve_only):
                pd, pi = pair_data[i], pair_in[i]
                if pd["qt"] is None:
                    pd["qt"] = bigsb.tile([128, T], FP16, tag="qt", name="qt")
                    pd["kt"] = bigsb.tile([128, T], FP16, tag="kt", name="kt")
                    pd["v_nat"] = bigsb.tile([128, T], FP16, tag="v_nat", name="v_nat")
                dst, wname, bname = (
                    (pd["qt"], "wq", "bq") if which == "q" else (pd["kt"], "wk", "bk")
                )
                psq = ps.tile([128, TQ], FP32, tag="sc", name="psq")
                for h in range(2):
                    sl = slice(h * 512, (h + 1) * 512)
                    tsl = slice(jj * TQ + h * 512, jj * TQ + (h + 1) * 512)
                    nc.tensor.matmul(
                        psq[:, sl], pd["w_h"][wname], pd["xgT"][:, tsl],
                        start=True, stop=True,
                    )
                dsl = slice(jj * TQ, (jj + 1) * TQ)
                if dve_only or jj % 2 == 0:
                    nc.vector.tensor_scalar_add(dst[:, dsl], psq, pi["b_col"][bname])
                else:
                    nc.scalar.add(dst[:, dsl], psq, pi["b_col"][bname])

            def emit_v_group(i, g, dve_only):
                pd = pair_data[i]
                c0 = g * 4
                psv = ps.tile([128, 512], FP32, tag="sc", name="psv")
                for q in range(4):
                    c = c0 + q
                    nc.tensor.matmul(
                        psv[:, q * 128 : (q + 1) * 128],
                        pd["xgT"][:, c * 128 : (c + 1) * 128], pd["w_h"]["wv"],
                        start=True, stop=True,
                    )
                vdsl = slice(c0 * 128, (c0 + 4) * 128)
                if dve_only or g % 2 == 1:
                    nc.vector.tensor_copy(pd["v_nat"][:, vdsl], psv)
                else:
                    nc.scalar.copy(pd["v_nat"][:, vdsl], psv)

            # --- pair 0 pre-attention prologue: xgT + early QK/V tiles ---
            # (w casts AFTER the xgT section: they wait on the weight DMA
            # chain and would otherwise block the DVE queue ahead of the
            # first slab cast)
            for k in range(4):
                emit_slab_cast(0, k)
                emit_T_group(0, 2 * k, False)
                emit_T_group(0, 2 * k + 1, False)
            emit_w_casts(0)
            emit_qk_tile(0, "q", 0, False)
            emit_qk_tile(0, "k", 0, False)
            emit_qk_tile(0, "k", 1, False)
            emit_v_group(0, 0, False)
            emit_v_group(0, 1, False)

            # --- filler queue: runs one item per attention chunk slot ---
            fillers = []
            # pair 0 leftovers (deadlines are early in macro 0; front-load)
            fillers += [
                lambda: emit_v_group(0, 2, True),
                lambda: emit_qk_tile(0, "k", 2, True),
                lambda: emit_v_group(0, 3, True),
                lambda: emit_v_group(0, 4, True),
                lambda: emit_qk_tile(0, "k", 3, True),
                lambda: emit_v_group(0, 5, True),
                lambda: emit_v_group(0, 6, True),
                lambda: emit_v_group(0, 7, True),
                lambda: emit_qk_tile(0, "q", 1, True),
                lambda: emit_qk_tile(0, "q", 2, True),
                lambda: emit_qk_tile(0, "q", 3, True),
            ]
            # pair 1 complete prologue
            def _p1_first():
                emit_w_casts(1)
                emit_slab_cast(1, 0)
            fillers.append(_p1_first)
            for k in range(4):
                if k > 0:
                    fillers.append(lambda k=k: emit_slab_cast(1, k))
                fillers.append(lambda k=k: emit_T_group(1, 2 * k, True))
                fillers.append(lambda k=k: emit_T_group(1, 2 * k + 1, True))
            for jj in range(4):
                fillers.append(lambda jj=jj: emit_qk_tile(1, "q", jj, True))
                fillers.append(lambda jj=jj: emit_qk_tile(1, "k", jj, True))
                fillers.append(lambda jj=jj: emit_v_group(1, 2 * jj, True))
                fillers.append(lambda jj=jj: emit_v_group(1, 2 * jj + 1, True))
            fillers.reverse()  # pop() from the end

            # ------------- attention: software-pipelined macro loop -------
            # prev: state of the previous macro whose epilogue is pending.
            prev = None

            def emit_epilogue_slot(c, st):
                """Emit the slice of the previous macro's epilogue assigned
                to chunk-slot c of the current macro stream."""
                if st is None:
                    return
                if c == 1:
                    # drain ps_out -> SBUF (DVE), reciprocal of den
                    nc.vector.tensor_copy(st["osb"], st["ps_out"])
                    nc.vector.reciprocal_approx_fast(
                        out=st["rcol"], in_=st["ps_den"]
                    )
                    if st.get("dbg") is not None:
                        dbg_sb = consts.tile([128, 16], FP32, tag="dbg_sb")
                        nc.vector.tensor_copy(dbg_sb[:, 0:8], st["ps_den"])
                        nc.vector.tensor_copy(dbg_sb[:, 8:16], st["rcol"])
                        st["dbg_sb"] = dbg_sb
                elif c in (3, 4):
                    j0 = 0 if c == 3 else 4
                    for j in range(j0, j0 + 4):
                        q = j if st.get("ep_wide") else j % 4
                        pse = st["ps_ep_t"]
                        nc.tensor.transpose(
                            pse[:, q * 128 : (q + 1) * 128],
                            st["osb"][:, j * 128 : (j + 1) * 128],
                            ident_h,
                        )
                        # out = (outT.T * 1/den) + bv  (bvb: partition-
                        # constant broadcast of bv, varies along e = free)
                        nc.vector.scalar_tensor_tensor(
                            out=st["out_sb"][:, j, :],
                            in0=pse[:, q * 128 : (q + 1) * 128],
                            scalar=st["rcol"][:, j : j + 1],
                            in1=st["bvb"],
                            op0=mybir.AluOpType.mult,
                            op1=mybir.AluOpType.add,
                        )
                elif c == 5:
                    nc.sync.dma_start(
                        out=st["y"][st["tq0"] : st["tq0"] + TQ, :].rearrange(
                            "(c p) d -> p c d", p=128
                        ),
                        in_=st["out_sb"],
                    )


            def emit_flush(st_cur, ptl, ppl):
                """Final out/den matmuls of the current macro (lag drain)."""
                for cc in (NCH - 2, NCH - 1):
                    for h in range(2):
                        sl = slice(h * 512, (h + 1) * 512)
                        nc.tensor.matmul(
                            st_cur["ps_out"][:, sl],
                            st_cur["v_nat"][:, cc * 128 : (cc + 1) * 128],
                            ptl[cc][:, sl],
                            start=(cc == 0), stop=(cc == NCH - 1),
                        )
                # NOTE: stop=True only on the final matmul touching the bank.
                # A matmul with start=False, stop=True whose write is NOT the
                # bank-final one loses its accumulation on TRN2 hardware
                # (observed: den columns 0..6 missing the last pair's sum).
                pp = NCH // 2 - 1  # last pair
                for j in range(8):
                    nc.tensor.matmul(
                        st_cur["ps_den"][:, j : j + 1],
                        ppl[pp][:, j * 128 : (j + 1) * 128],
                        ones_col,
                        start=(pp == 0), stop=(j == 7),
                    )

            for i in range(PAIRS_PER_CORE):
                pd = pair_data[i]
                qt, kt, v_nat = pd["qt"], pd["kt"], pd["v_nat"]
                assert qt is not None, f"pair {i} prologue tiles not emitted yet"
                for m in range(NM):
                    tq0 = m * TQ
                    ps_out = ps_o.tile([128, TQ], FP32, tag="o")
                    ps_den = ps_dn.tile([128, 8], FP32, tag="dn")
                    ptl = [None] * NCH
                    ppl = [None] * (NCH // 2)
                    st_cur = dict(
                        ps_out=ps_out, ps_den=ps_den, v_nat=v_nat, tq0=tq0,
                        y=outs[i], bvb=pd["bvb"],
                        dbg=outs_dbg if (i == 1 and m == NM - 1) else None,
                    )
                    for c in range(NCH):
                        # scores for chunk c
                        sc_t = ps.tile([128, TQ], FP32, tag="sc")
                        ksl = kt[:, c * 128 : (c + 1) * 128]
                        for h in range(2):
                            sl = slice(h * 512, (h + 1) * 512)
                            qsl = slice(tq0 + h * 512, tq0 + (h + 1) * 512)
                            nc.tensor.matmul(
                                sc_t[:, sl], ksl, qt[:, qsl], start=True, stop=True
                            )
                        pt_c = ptp.tile([128, TQ], FP16, tag="pt")
                        nc.scalar.activation(
                            pt_c, sc_t, mybir.ActivationFunctionType.Exp,
                            scale=INV_SCALE,
                        )
                        ptl[c] = pt_c

                        # previous macro's epilogue rides in early slots
                        emit_epilogue_slot(c, prev)

                        # lagged out-matmuls (chunk c-2)
                        if c >= 2:
                            cc = c - 2
                            for h in range(2):
                                sl = slice(h * 512, (h + 1) * 512)
                                nc.tensor.matmul(
                                    ps_out[:, sl], v_nat[:, cc * 128 : (cc + 1) * 128], ptl[cc][:, sl],
                                    start=(cc == 0), stop=False,
                                )

                        # one prologue filler per chunk slot
                        if fillers:
                            fillers.pop()()

                        # chunk-pair sum; lagged den matmuls
                        if c % 2 == 1:
                            pp_i = (c - 1) // 2
                            pp_t = ppair.tile([128, TQ], FP16, tag="pp")
                            nc.vector.tensor_add(pp_t, ptl[c - 1], ptl[c])
                            ppl[pp_i] = pp_t
                            if c >= 3:
                                dp = (c - 3) // 2
                                for j in range(8):
                                    # start=True ONLY on the very first
                                    # column-matmul: a start marks the whole
                                    # 2KB psum zero-region pending-zero, so
                                    # start on later columns would discard
                                    # the columns written just before.
                                    nc.tensor.matmul(
                                        ps_den[:, j : j + 1],
                                        ppl[dp][:, j * 128 : (j + 1) * 128],
                                        ones_col,
                                        start=(dp == 0 and j == 0), stop=False,
                                    )
                    # flush the lag of this macro
                    emit_flush(st_cur, ptl, ppl)
                    # hand off epilogue state
                    osb = epi.tile([128, TQ], FP16, tag="osb")
                    rcol = epi.tile([128, 8], FP32, tag="rcol")
                    out_sb = epi.tile([128, TQ // 128, 128], FP32, tag="out_sb")
                    ps_ep_t = ps_ep.tile([128, 512], FP16, tag="ep")
                    st_cur.update(osb=osb, rcol=rcol, out_sb=out_sb, ps_ep_t=ps_ep_t)
                    prev = st_cur

            # tail: last macro's epilogue, emitted standalone; the sc
            # pool slots are free now -- transpose all 8 blocks into two
            # [128,512] tiles first, then run all the normalizing muls, so
            # the PE never waits on the DVE.
            emit_epilogue_slot(1, prev)
            tails = []
            for half in range(2):
                pse = ps.tile([128, 512], FP16, tag="sc", name="ep_tail")
                for q in range(4):
                    j = half * 4 + q
                    nc.tensor.transpose(
                        pse[:, q * 128 : (q + 1) * 128],
                        prev["osb"][:, j * 128 : (j + 1) * 128],
                        ident_h,
                    )
                tails.append(pse)
            for half in range(2):
                for q in range(4):
                    j = half * 4 + q
                    nc.vector.scalar_tensor_tensor(
                        out=prev["out_sb"][:, j, :],
                        in0=tails[half][:, q * 128 : (q + 1) * 128],
                        scalar=prev["rcol"][:, j : j + 1],
                        in1=prev["bvb"],
                        op0=mybir.AluOpType.mult,
                        op1=mybir.AluOpType.add,
                    )
            emit_epilogue_slot(5, prev)
            if prev.get("dbg") is not None:
                # late re-read of the final macro's den psum, after everything
                nc.vector.tensor_copy(prev["dbg_sb"][:, 8:16], prev["ps_den"])
                nc.sync.dma_start(out=prev["dbg"][:, :], in_=prev["dbg_sb"])

    # populate .instr bytes for extended InstISA ops (custom DVE reciprocal);
    # raw Bass skips this pass and walrus then fails with "ISA wrong length".
    mybir.codegen_inst_isa_subclasses(nc)
    _split_multi_waits(nc)
    return nc


def _get_nc():
    global _NC_CACHE
    if _NC_CACHE is None:
        _NC_CACHE = build_nc()
    return _NC_CACHE


def kernel(**inputs: np.ndarray) -> np.ndarray:
    x = np.ascontiguousarray(inputs["x"], dtype=np.float32)
    Wq = np.asarray(inputs["Wq"], dtype=np.float32)
    Wk = np.asarray(inputs["Wk"], dtype=np.float32)
    Wv = np.asarray(inputs["Wv"], dtype=np.float32)
    bq = np.asarray(inputs["bq"], dtype=np.float32)
    bk = np.asarray(inputs["bk"], dtype=np.float32)
    bv = np.asarray(inputs["bv"], dtype=np.float32)

    nc = _get_nc()

    in_maps = []
    for core in range(N_CORES):
        m = {}
        for i in range(PAIRS_PER_CORE):
            pair = core * PAIRS_PER_CORE + i
            b, g = pair // G, pair % G
            sl = slice(g * GS, (g + 1) * GS)
            m[f"x{i}"] = np.ascontiguousarray(x[b, :, sl])
            m[f"wq{i}"] = np.ascontiguousarray(Wq[g])
            m[f"wk{i}"] = np.ascontiguousarray(Wk[g])
            m[f"wv{i}"] = np.ascontiguousarray(Wv[g])
            m[f"bq{i}"] = np.ascontiguousarray(bq[g].reshape(1, GS))
            m[f"bk{i}"] = np.ascontiguousarray(bk[g].reshape(1, GS))
            m[f"bv{i}"] = np.ascontiguousarray(bv[g].reshape(1, GS))
        in_maps.append(m)

    global _LAST_IN_MAPS
    _LAST_IN_MAPS = in_maps

    from concourse.bass_utils import run_bass_kernel_spmd

    res = run_bass_kernel_spmd(nc, in_maps, list(range(N_CORES)))

    y = np.empty((B, T, F), dtype=np.float32)
    for core in range(N_CORES):
        for i in range(PAIRS_PER_CORE):
            pair = core * PAIRS_PER_CORE + i
            b, g = pair // G, pair % G
            y[b, :, g * GS : (g + 1) * GS] = res.results[core][f"y{i}"]
    return y
